# revision 1
# baseline (speedup 1.0000x reference)
"""MSE-style custom loss on 8 Trainium2 NeuronCores.

reference: d = |input - target|; conditional 0.8 scale of d[0] when
d[0] in {3,4,5,6}; return mean(d*d).

Strategy (data-parallel, memory-bound):
  - Split the 32M-element 1-D tensors into 8 contiguous shards (4M each).
  - Per core: stream [128 x F] fp32 tiles of both operands from DRAM,
    d = a - b on the vector engine, then Square activation on the scalar
    engine with accum_out -> per-partition partial sums (one column per
    compute slice).  2 compute ops per element; both engines pipeline
    well under the DMA roofline (~32 MiB/core, measured ~370-410 GB/s
    sustained with 32 KB descriptors -> ~82-90 us streaming).
  - Host: sum the 8 x [128 x n_cols] partials in f64, apply the d[0]
    fixup (only touches one element), divide by N.
"""

import numpy as np

N = 33554432
N_CORES = 8
SHARD = N // N_CORES          # 4194304
P = 128
# Chunk free-dims.  Big 4 MiB body tiles (32 KB DMA descriptors per
# partition row) for bandwidth; progressively smaller tail tiles so the
# trailing compute after the last DMA byte is short.  Compute runs in
# <=SLICE-wide sub-slices so the scalar engine pipelines behind the
# vector engine and pool slots release early.
BODY = [8192, 8192, 8192]
TAIL = [2048, 2048, 2048, 1024, 512, 512]
SLICE = 2048
assert (sum(BODY) + sum(TAIL)) * P == SHARD

_cache = {}


def _get_program():
    if "nc" in _cache:
        return _cache["nc"]

    import concourse.tile as tile
    from concourse import bacc, mybir

    nc = bacc.Bacc("TRN2", target_bir_lowering=False, debug=False)
    a_d = nc.dram_tensor("input", [SHARD], mybir.dt.float32,
                         kind="ExternalInput").ap()
    b_d = nc.dram_tensor("target", [SHARD], mybir.dt.float32,
                         kind="ExternalInput").ap()
    body_cols = sum(max(1, f // SLICE) for f in BODY)
    tail_cols = sum(max(1, f // SLICE) for f in TAIL)
    n_cols = body_cols + tail_cols
    out_d = nc.dram_tensor("partial", [P, n_cols], mybir.dt.float32,
                           kind="ExternalOutput").ap()

    def chunk_ap(base, off, f):
        return base[off:off + P * f].rearrange("(p f) -> p f", p=P, f=f)

    with tile.TileContext(nc) as tc:
        with tc.tile_pool(name="a", bufs=2) as pa, \
             tc.tile_pool(name="b", bufs=2) as pb, \
             tc.tile_pool(name="at", bufs=3) as pat, \
             tc.tile_pool(name="bt", bufs=3) as pbt, \
             tc.tile_pool(name="acc", bufs=1) as pacc:
            acc = pacc.tile([P, n_cols], mybir.dt.float32)
            off = 0
            col = 0
            for f in BODY + TAIL:
                tail = f <= SLICE
                ta = (pat if tail else pa).tile([P, f], mybir.dt.float32,
                                                tag="at" if tail else "a")
                nc.sync.dma_start(ta[:], chunk_ap(a_d, off, f))
                tb = (pbt if tail else pb).tile([P, f], mybir.dt.float32,
                                                tag="bt" if tail else "b")
                nc.sync.dma_start(tb[:], chunk_ap(b_d, off, f))
                for s in range(0, f, SLICE):
                    w = min(SLICE, f - s)
                    nc.vector.tensor_sub(ta[:, s:s + w], ta[:, s:s + w],
                                         tb[:, s:s + w])
                    nc.scalar.activation(ta[:, s:s + w], ta[:, s:s + w],
                                         mybir.ActivationFunctionType.Square,
                                         accum_out=acc[:, col:col + 1])
                    col += 1
                off += P * f
            assert col == n_cols
            # Issued from the scalar engine: program-order after the last
            # Square on the same engine, so no cross-engine sem hop.
            nc.scalar.dma_start(out_d[:], acc[:])

    nc.compile()
    _cache["nc"] = nc
    return nc


def run_spmd(input, target, trace=False, **kw):
    """Run the sharded kernel; returns (partial_sums_f64, BassKernelResults)."""
    from concourse.bass_utils import run_bass_kernel_spmd

    nc = _get_program()
    a = np.ascontiguousarray(np.asarray(input, dtype=np.float32)
                             ).reshape(N_CORES, SHARD)
    b = np.ascontiguousarray(np.asarray(target, dtype=np.float32)
                             ).reshape(N_CORES, SHARD)
    in_maps = [{"input": a[c], "target": b[c]} for c in range(N_CORES)]
    br = None
    delays = [3.0, 10.0, 20.0]
    for attempt in range(len(delays) + 1):
        try:
            br = run_bass_kernel_spmd(nc, in_maps, list(range(N_CORES)),
                                      trace=trace, **kw)
            break
        except Exception:
            # Transient NRT/device hiccups (e.g. NRT_EXEC_UNIT_UNRECOVERABLE)
            # clear on retry.
            if attempt == len(delays):
                raise
            import time
            time.sleep(delays[attempt])
    total = 0.0
    for r in br.results:
        total += float(np.sum(r["partial"], dtype=np.float64))
    return total, br


def kernel(input, target):
    input = np.asarray(input)
    target = np.asarray(target)
    total, _ = run_spmd(input, target)

    # res[0] fixup, faithful to the fp32 reference semantics.
    d0 = np.float32(abs(np.float32(input.reshape(-1)[0]) -
                        np.float32(target.reshape(-1)[0])))
    if d0 in (np.float32(3.0), np.float32(4.0),
              np.float32(5.0), np.float32(6.0)):
        d0f = np.float32(d0 * np.float32(0.8))
        total += float(d0f) * float(d0f) - float(d0) * float(d0)

    return np.array(total / N, dtype=np.float32)



# revision 2
# speedup vs baseline: 1.2398x; 1.2398x over previous
"""MSE-style custom loss on 8 Trainium2 NeuronCores — raw-bacc version.

reference: d = |input - target|; conditional 0.8 scale of d[0] when
d[0] in {3,4,5,6}; return mean(d*d).

Strategy (data-parallel, memory-bound):
  - Split the 32M-element tensors into 8 contiguous shards (4M each).
  - Host-side, interleave input/target per chunk so each core streams ONE
    contiguous DRAM tensor "ab": per chunk of f cols, a [128, 2f] region
    whose partition rows hold f input elems then f target elems.
  - Raw bacc program (no TileContext): hand-placed semaphores.  Body
    chunks (7 x 4 MiB) rotate through 3 SBUF buffers; a geometric tail
    (2 MiB .. 16 KiB) gets dedicated buffers so no tail DMA waits.  All
    loads issue on the sync HWDGE ring as large contiguous transfers
    (sustains ~425 GB/s/core, near the 435 GB/s fabric ceiling).
  - Compute pipelines behind the stream in <=2048-col slices:
    vector tensor_sub -> scalar Square activation with accum_out
    per-partition column sums.
  - Host: sum the 8 x [128, n_cols] partials in f64, apply the d[0]
    fixup, divide by N.
"""

import numpy as np

N = 33554432
N_CORES = 8
SHARD = N // N_CORES          # 4194304
P = 128
BODY = [4096] * 7             # 3 rotating buffers, WAR wait distance 3
TAIL = [2048, 1024, 512, 256, 128, 64, 32, 32]   # dedicated buffer each
CHUNKS = BODY + TAIL
N_BODY_BUFS = 3
SLICE_W = 2048
assert sum(CHUNKS) * P == SHARD

_cache = {}


def _get_program():
    if "nc" in _cache:
        return _cache["nc"]

    from contextlib import ExitStack

    from concourse import bacc, mybir

    nc = bacc.Bacc("TRN2", target_bir_lowering=False, debug=False)
    ab_d = nc.dram_tensor("ab", [2 * SHARD], mybir.dt.float32,
                          kind="ExternalInput").ap()
    n_cols = sum(max(1, f // SLICE_W) for f in CHUNKS)
    out_d = nc.dram_tensor("partial", [P, n_cols], mybir.dt.float32,
                           kind="ExternalOutput").ap()

    with ExitStack() as st:
        body_bufs = [
            st.enter_context(nc.sbuf_tensor(
                f"bb{i}", [P, 2 * BODY[0]], mybir.dt.float32))
            for i in range(N_BODY_BUFS)
        ]
        tail_bufs = [
            st.enter_context(nc.sbuf_tensor(
                f"tb{i}", [P, 2 * f], mybir.dt.float32))
            for i, f in enumerate(TAIL)
        ]
        acc = st.enter_context(nc.sbuf_tensor(
            "acc", [P, n_cols], mybir.dt.float32))
        in_sems = [st.enter_context(nc.semaphore(f"in{i}"))
                   for i in range(len(CHUNKS))]
        sem_v = st.enter_context(nc.semaphore("semv"))
        sem_sq = st.enter_context(nc.semaphore("semsq"))
        sem_out = st.enter_context(nc.semaphore("semout"))

        nb = len(BODY)
        off = 0
        for j, f in enumerate(CHUNKS):
            buf = body_bufs[j % N_BODY_BUFS] if j < nb else tail_bufs[j - nb]
            if j < nb and j >= N_BODY_BUFS:
                # WAR: buffer reused from chunk j-3; its last reader is the
                # scalar Square (sem_sq inc per finished chunk).
                nc.sync.wait_ge(sem_sq, j - N_BODY_BUFS + 1)
            nc.sync.dma_start(
                buf[:, 0:2 * f],
                ab_d[off:off + P * 2 * f].rearrange(
                    "(p f) -> p f", p=P, f=2 * f),
            ).then_inc(in_sems[j], 16)
            off += P * 2 * f
        col = 0
        gslice = 0
        for j, f in enumerate(CHUNKS):
            buf = body_bufs[j % N_BODY_BUFS] if j < nb else tail_bufs[j - nb]
            nc.vector.wait_ge(in_sems[j], 16)
            n_sl = max(1, f // SLICE_W)
            for si in range(n_sl):
                s = si * SLICE_W
                w = min(SLICE_W, f - s)
                nc.vector.tensor_sub(buf[:, s:s + w], buf[:, s:s + w],
                                     buf[:, f + s:f + s + w]).then_inc(
                                         sem_v, 1)
                gslice += 1
                nc.scalar.wait_ge(sem_v, gslice)
                ins = nc.scalar.activation(
                    buf[:, s:s + w], buf[:, s:s + w],
                    mybir.ActivationFunctionType.Square,
                    accum_out=acc[:, col:col + 1])
                if si == n_sl - 1:
                    ins.then_inc(sem_sq, 1)
                col += 1
        assert col == n_cols
        # drain: the last ACTIVATION_READ_ACCUMULATOR must retire before
        # the out DMA's descriptors read acc.
        nc.scalar.drain(fusable=False)
        nc.scalar.dma_start(out_d[:], acc[:]).then_inc(sem_out, 16)
        nc.scalar.wait_ge(sem_out, 16)
    nc.compile()
    _cache["nc"] = nc
    return nc


def _interleave(a, b):
    """Per-core merged layout: for each chunk f, [128, 2f] with input in
    cols :f and target in f:.  Returns list of 8 flat arrays."""
    ar = a.reshape(N_CORES, SHARD)
    br = b.reshape(N_CORES, SHARD)
    out = np.empty((N_CORES, 2 * SHARD), dtype=np.float32)
    for c in range(N_CORES):
        off = 0
        doff = 0
        for f in CHUNKS:
            n = P * f
            blk = out[c, doff:doff + 2 * n].reshape(P, 2 * f)
            blk[:, :f] = ar[c, off:off + n].reshape(P, f)
            blk[:, f:] = br[c, off:off + n].reshape(P, f)
            off += n
            doff += 2 * n
    return out


def run_spmd(input, target, trace=False, **kw):
    """Run the sharded kernel; returns (partial_sums_f64, BassKernelResults)."""
    from concourse.bass_utils import run_bass_kernel_spmd

    nc = _get_program()
    a = np.ascontiguousarray(np.asarray(input, dtype=np.float32))
    b = np.ascontiguousarray(np.asarray(target, dtype=np.float32))
    ab = _interleave(a.reshape(-1), b.reshape(-1))
    in_maps = [{"ab": ab[c]} for c in range(N_CORES)]
    br = None
    delays = [3.0, 10.0, 20.0]
    for attempt in range(len(delays) + 1):
        try:
            br = run_bass_kernel_spmd(nc, in_maps, list(range(N_CORES)),
                                      trace=trace, **kw)
            break
        except Exception:
            # Transient NRT/device hiccups clear on retry.
            if attempt == len(delays):
                raise
            import time
            time.sleep(delays[attempt])
    total = 0.0
    for r in br.results:
        total += float(np.sum(r["partial"], dtype=np.float64))
    return total, br


def kernel(input, target):
    input = np.asarray(input)
    target = np.asarray(target)
    total, _ = run_spmd(input, target)

    # res[0] fixup, faithful to the fp32 reference semantics.
    d0 = np.float32(abs(np.float32(input.reshape(-1)[0]) -
                        np.float32(target.reshape(-1)[0])))
    if d0 in (np.float32(3.0), np.float32(4.0),
              np.float32(5.0), np.float32(6.0)):
        d0f = np.float32(d0 * np.float32(0.8))
        total += float(d0f) * float(d0f) - float(d0) * float(d0)

    return np.array(total / N, dtype=np.float32)
